# revision 29
# baseline (speedup 1.0000x reference)
"""DSFusion kernel for 8x TRN2 NeuronCores.

Computation (per reference):
    out_x = x @ Wx.T + bx ; out_y = y @ Wy.T + by
    sp1 = softplus(out_x) ; sp2 = softplus(out_y)
    alpha_x = sp1 + 1 ; alpha_y = sp2 + 1
    alpha_a = sp1*sp2/C + sp1 + sp2 + 1        (algebraic collapse of the
                                                Dempster-Shafer combination --
                                                all S/b/u/conflict terms cancel)

Sharding: data-parallel over the batch dim, 1024 rows per core; weights and
biases replicated. Host pre-transposes x/y/W so the contraction dim sits on
SBUF partitions and pre-casts matmul operands to bf16 (fp32 PSUM accumulate).

Schedule: rows are processed in 4 pairs of 128-row tiles. Per pair, an
X-phase accumulates out_x for both row tiles over all 16 K-chunks (4 PSUM
banks), spills psum+bias to SBUF, then a Y-phase does the same for out_y.
x/y are laid out host-side as contiguous [K-chunk, pair] tiles and DMAs are
issued in first-touch order, so the PE starts within ~2us and never waits
on the bulk weight load.
"""

import numpy as np
import ml_dtypes

BATCH = 8192
DIM = 2048
CLASSES = 1000
NCORES = 8
R = BATCH // NCORES          # rows per core (1024)
P = 128
KCH = DIM // P               # contraction chunks (16)
NPAIR = 4                    # pairs of 128-row tiles per core
PW = 2 * P                   # rows per pair (256)
NH = CLASSES // 2            # psum half (500, fits one 2KB bank)

_CACHE = {}

# Results of the last device run (for the test harness to inspect timing).
LAST_RESULTS = None


def _split_waits(nc, limit=1):
    """The installed walrus can't lower an instruction carrying more than one
    sync wait. Hoist extra waits onto single-wait NOPs inserted immediately
    before the instruction on the same engine (program order preserves the
    wait-all semantics)."""
    import concourse.mybir as mybir

    for f in nc.m.functions:
        for bb in f.blocks:
            out = []
            changed = False
            for ins in list(bb.instructions):
                si = ins.sync_info
                if si is not None and len(si.on_wait) > limit:
                    waits = list(si.on_wait)
                    extra, keep = waits[:-limit], waits[-limit:]
                    for i, w in enumerate(extra):
                        nop = mybir.InstNoOp(name=f"{ins.name}-ws{i}", ins=[], outs=[])
                        nop.engine = ins.engine
                        nop.sync_info = mybir.SyncInfo(on_wait=[w], on_update=[])
                        out.append(nop)
                    ins.sync_info = mybir.SyncInfo(
                        on_wait=keep, on_update=list(si.on_update)
                    )
                    changed = True
                out.append(ins)
            if changed:
                bb.instructions = out


def _build_nc():
    import concourse.bass as bass
    import concourse.mybir as mybir
    import concourse.tile as tile
    from concourse.vector_clock import ScopedClock, VectorClock

    class LeanTailTileContext(tile.TileContext):
        """Tile's stock tail is drain + two all-engine barriers + sem clears;
        with the single-wait-per-instruction legalization the barrier waits
        explode into a ~10us serial EVSEM parade. Replace with: SP drain
        (single-wait NOPs), a two-semaphore handshake barrier (one wait per
        engine), then gpsimd range-clears everything last."""

        def _drain_and_barrier(self, tick_clock, wait_clock):
            nc = self.nc
            vc = tick_clock.global_clock
            n = len(vc)
            for proc in range(n):
                t = vc[proc]
                if t > 0:
                    nop = nc.sync.nop(nofuse=True, hint=f"tail_wait_{proc}")
                    req = ScopedClock(
                        {None: VectorClock([t if i == proc else 0 for i in range(n)])}
                    )
                    wait_clock.add_sem_waits(nop.ins, req)
            nc.sync.drain()

            semB = nc.alloc_semaphore("tail_barrier_b")
            semC = nc.alloc_semaphore("tail_barrier_c")
            engines = list(nc.engines.values())
            pool_eng = nc.gpsimd
            n_eng = len(engines)
            for e in engines:
                e.nop(nofuse=True, hint="tailb_inc").then_inc(semB, 1)
            for e in engines:
                e.wait_ge(semB, n_eng)
            for e in engines:
                if e is not pool_eng:
                    e.nop(nofuse=True, hint="tailc_inc").then_inc(semC, 1)
            pool_eng.wait_ge(semC, n_eng - 1)

            assert self.sems is not None
            popped = self.nc._tile_sem_poison_stack.pop()
            assert popped is self._sem_poison
            nc.clear_and_free_semaphores(
                list(self.sems.allocated().values()) + [semB, semC]
            )

    dt = mybir.dt

    nc = bass.Bass()

    xT = nc.dram_tensor("xT", [DIM, R], dt.bfloat16, kind="ExternalInput")
    yT = nc.dram_tensor("yT", [DIM, R], dt.bfloat16, kind="ExternalInput")
    wxT = nc.dram_tensor("wxT", [DIM, CLASSES], dt.bfloat16, kind="ExternalInput")
    wyT = nc.dram_tensor("wyT", [DIM, CLASSES], dt.bfloat16, kind="ExternalInput")
    bxb = nc.dram_tensor("bxb", [P, CLASSES], dt.bfloat16, kind="ExternalInput")
    byb = nc.dram_tensor("byb", [P, CLASSES], dt.bfloat16, kind="ExternalInput")

    aa_d = nc.dram_tensor("alpha_a", [R, CLASSES], dt.float32, kind="ExternalOutput")
    ax_d = nc.dram_tensor("alpha_x", [R, CLASSES], dt.float32, kind="ExternalOutput")
    ay_d = nc.dram_tensor("alpha_y", [R, CLASSES], dt.float32, kind="ExternalOutput")

    xT3 = xT.rearrange("(ko p) r -> p ko r", p=P)
    yT3 = yT.rearrange("(ko p) r -> p ko r", p=P)
    wxT3 = wxT.rearrange("(ko p) c -> p ko c", p=P)
    wyT3 = wyT.rearrange("(ko p) c -> p ko c", p=P)
    aa3 = aa_d.rearrange("(t p) c -> t p c", p=P)
    ax3 = ax_d.rearrange("(t p) c -> t p c", p=P)
    ay3 = ay_d.rearrange("(t p) c -> t p c", p=P)

    # softplus(x) = ln(exp(x) + 1); the installed ACT tables have no direct
    # softplus, but exp and ln share one table set. Pre-activation values are
    # within +-4 so exp cannot overflow.
    EXP = mybir.ActivationFunctionType.Exp
    LN = mybir.ActivationFunctionType.Ln
    ADD = mybir.AluOpType.add
    MULT = mybir.AluOpType.mult

    with LeanTailTileContext(nc) as tc:
        with (
            tc.tile_pool(name="wpool", bufs=1) as wpool,
            tc.tile_pool(name="xpool", bufs=1) as xpool,
            tc.tile_pool(name="epool", bufs=2) as epool,
            tc.tile_pool(name="opool", bufs=1) as opool,
            tc.tile_pool(name="psum", bufs=1, space="PSUM") as ppool,
        ):
            # -- input DMAs ------------------------------------------------
            # x/wx trigger from the SP sequencer, y/wy from the ACT
            # sequencer (both HWDGE), 2 K-chunks per DMA. The DMA engines
            # round-robin descriptors across all in-flight transfers, so
            # every transfer is gated (add_dep_helper below) on the PE
            # reaching the matmul ~2 K-groups before its first use --
            # otherwise the bulk floods the engines and the ramp data
            # arrives an order of magnitude late.
            x0_sb, y0_sb, wx_sb, wy_sb = [], [], [], []
            x0_dma, y0_dma, wx_dma, wy_dma = [], [], [], []
            for kk in range(KCH // 2):
                t_ = xpool.tile([P, 2, PW], dt.bfloat16, tag=f"x0_{kk}")
                x0_dma.append(nc.sync.dma_start(t_[:], xT3[:, 2 * kk:2 * kk + 2, 0:PW]))
                x0_sb.append(t_)
                t_ = wpool.tile([P, 2, CLASSES], dt.bfloat16, tag=f"wx{kk}")
                wx_dma.append(nc.sync.dma_start(t_[:], wxT3[:, 2 * kk:2 * kk + 2, :]))
                wx_sb.append(t_)
                t_ = xpool.tile([P, 2, PW], dt.bfloat16, tag=f"y0_{kk}")
                y0_dma.append(nc.scalar.dma_start(t_[:], yT3[:, 2 * kk:2 * kk + 2, 0:PW]))
                y0_sb.append(t_)
                t_ = wpool.tile([P, 2, CLASSES], dt.bfloat16, tag=f"wy{kk}")
                wy_dma.append(nc.scalar.dma_start(t_[:], wyT3[:, 2 * kk:2 * kk + 2, :]))
                wy_sb.append(t_)

            bx_sb = wpool.tile([P, CLASSES], dt.bfloat16, tag="bx")
            bx_dma = nc.sync.dma_start(bx_sb[:], bxb[:])
            by_sb = wpool.tile([P, CLASSES], dt.bfloat16, tag="by")
            by_dma = nc.scalar.dma_start(by_sb[:], byb[:])

            # row tiles 2..7 bulk: [2 K-chunks x 6 row tiles x P] per DMA
            x1_sb, y1_sb = [], []
            x1_dma, y1_dma = [], []
            for kk in range(KCH // 2):
                t_ = xpool.tile([P, 2, (NPAIR - 1) * PW], dt.bfloat16, tag=f"x{kk}_b")
                x1_dma.append(nc.sync.dma_start(t_[:], xT3[:, 2 * kk:2 * kk + 2, PW:R]))
                x1_sb.append(t_)
                t_ = xpool.tile([P, 2, (NPAIR - 1) * PW], dt.bfloat16, tag=f"y{kk}_b")
                y1_dma.append(nc.scalar.dma_start(t_[:], yT3[:, 2 * kk:2 * kk + 2, PW:R]))
                y1_sb.append(t_)

            def x_slice(k, r):  # lhsT for global row tile r, K-chunk k
                if r < 2:
                    return x0_sb[k // 2][:, k % 2, r * P:(r + 1) * P]
                return x1_sb[k // 2][:, k % 2, (r - 2) * P:(r - 1) * P]

            def y_slice(k, r):
                if r < 2:
                    return y0_sb[k // 2][:, k % 2, r * P:(r + 1) * P]
                return y1_sb[k // 2][:, k % 2, (r - 2) * P:(r - 1) * P]

            def wx_slice(k, hs):
                return wx_sb[k // 2][:, k % 2, hs]

            def wy_slice(k, hs):
                return wy_sb[k // 2][:, k % 2, hs]

            HS = [slice(0, NH), slice(NH, CLASSES)]

            # PE warm-up: HAM starts at half clock and re-throttles after
            # idle; ~4.5us of dummy matmuls during the DMA prefix brings the
            # PE to K=8/8 before the first real matmul.
            wl = xpool.tile([P, P], dt.bfloat16, tag="warm_l")
            nc.vector.memset(wl[:], 0)
            wr = xpool.tile([P, NH], dt.bfloat16, tag="warm_r")
            nc.vector.memset(wr[:], 0)
            wp = ppool.tile([P, NH], dt.float32, tag="ps0_0", name="warmp")
            for _ in range(20):
                nc.tensor.matmul(wp[:], wl[:], wr[:], start=True, stop=True)

            # -- compute ---------------------------------------------------
            # Row tiles are processed in units of [2,2,2,1,1]; per unit an
            # X phase accumulates out_x over all K-chunks (2 PSUM banks per
            # row tile), its epilogue overlaps the Y phase. The small final
            # units keep the tail-exposed epilogue short.
            from concourse.tile_rust import add_dep_helper

            UNITS = [(0, 2), (2, 2), (4, 2), (6, 1), (7, 1)]
            YPRE = 4  # Y-phase K-groups pre-issued into unit 0's X phase
            mm_anchor = {}  # (phase_idx, k) -> last MM instruction

            phase_idx = 0
            for u, (r0, nrt) in enumerate(UNITS):
                # ---- X phase ----
                psx = [
                    [ppool.tile([P, NH], dt.float32, tag=f"ps{jj}_{h}", name=f"ps{jj}_{h}") for h in range(2)]
                    for jj in range(nrt)
                ]
                for k in range(KCH):
                    st, sp = k == 0, k == KCH - 1
                    for jj in range(nrt):
                        lhsT = x_slice(k, r0 + jj)
                        nc.tensor.matmul(psx[jj][0][:], lhsT, wx_slice(k, HS[0]), start=st, stop=sp)
                        mm = nc.tensor.matmul(psx[jj][1][:], lhsT, wx_slice(k, HS[1]), start=st, stop=sp)
                    mm_anchor[(phase_idx, k)] = mm.ins
                phase_idx += 1

                # Pre-issue the first K-groups of unit 0's Y phase so the PE
                # has work while the Y-phase ramp data lands (fills the
                # measured ~3us X0->Y0 boundary gap).
                psy = None
                if u == 0:
                    psy = [
                        [ppool.tile([P, NH], dt.float32, tag=f"ps{4 + jj}_{h}", name=f"ps{4 + jj}_{h}") for h in range(2)]
                        for jj in range(nrt)
                    ]
                    for k in range(YPRE):
                        st = k == 0
                        for jj in range(nrt):
                            lhsT = y_slice(k, r0 + jj)
                            nc.tensor.matmul(psy[jj][0][:], lhsT, wy_slice(k, HS[0]), start=st, stop=False)
                            mm = nc.tensor.matmul(psy[jj][1][:], lhsT, wy_slice(k, HS[1]), start=st, stop=False)
                        mm_anchor[(phase_idx, k)] = mm.ins

                t1 = []
                for jj in range(nrt):
                    t_ = epool.tile([P, CLASSES], dt.float32, tag=f"t1_{jj}")
                    nc.vector.tensor_tensor(t_[:, HS[0]], psx[jj][0][:], bx_sb[:, HS[0]], ADD)
                    nc.vector.tensor_tensor(t_[:, HS[1]], psx[jj][1][:], bx_sb[:, HS[1]], ADD)
                    t1.append(t_)

                # X epilogue overlaps the Y phase: softplus, alpha_x,
                # then w1 = sp1/C + 1 in place of t1.
                axt = []
                for jj in range(nrt):
                    r = r0 + jj
                    ax = opool.tile([P, CLASSES], dt.float32, tag=f"ax{jj}")
                    for h in range(2):
                        hs = HS[h]
                        sp1 = t1[jj][:, hs]
                        nc.scalar.activation(sp1, sp1, EXP)
                        nc.scalar.activation(sp1, sp1, LN, bias=1.0)
                        nc.vector.tensor_scalar_add(ax[:, hs], sp1, 1.0)
                        nc.sync.dma_start(ax3[r][:, hs], ax[:, hs])
                        nc.vector.tensor_scalar(sp1, sp1, 1.0 / CLASSES, 1.0, MULT, ADD)
                    axt.append(ax)

                # ---- Y phase ----
                k_start = 0
                if psy is None:
                    psy = [
                        [ppool.tile([P, NH], dt.float32, tag=f"ps{4 + jj}_{h}", name=f"ps{4 + jj}_{h}") for h in range(2)]
                        for jj in range(nrt)
                    ]
                else:
                    k_start = YPRE
                if u == len(UNITS) - 1:
                    # Last unit: group the Y matmuls by column half and run
                    # the (tail-exposed) epilogue per half, so half 0's
                    # epilogue and output DMAs overlap half 1's matmuls.
                    r = r0
                    t_ = epool.tile([P, CLASSES], dt.float32, tag="t2_0")
                    ay = opool.tile([P, CLASSES], dt.float32, tag="ay0")
                    aa = opool.tile([P, CLASSES], dt.float32, tag="aa0")
                    for h in range(2):
                        hs = HS[h]
                        for k in range(KCH):
                            st, sp = k == 0, k == KCH - 1
                            mm = nc.tensor.matmul(psy[0][h][:], y_slice(k, r), wy_slice(k, hs), start=st, stop=sp)
                            if h == 0:
                                mm_anchor[(phase_idx, k)] = mm.ins
                        sp2 = t_[:, hs]
                        nc.vector.tensor_tensor(sp2, psy[0][h][:], by_sb[:, hs], ADD)
                        nc.scalar.activation(sp2, sp2, EXP)
                        nc.scalar.activation(sp2, sp2, LN, bias=1.0)
                        nc.scalar.add(ay[:, hs], sp2, 1.0)
                        nc.sync.dma_start(ay3[r][:, hs], ay[:, hs])
                        nc.vector.tensor_tensor(sp2, sp2, t1[0][:, hs], MULT)
                        nc.vector.tensor_tensor(aa[:, hs], sp2, axt[0][:, hs], ADD)
                        nc.sync.dma_start(aa3[r][:, hs], aa[:, hs])
                    phase_idx += 1
                    continue
                for k in range(k_start, KCH):
                    st, sp = k == 0, k == KCH - 1
                    for jj in range(nrt):
                        lhsT = y_slice(k, r0 + jj)
                        nc.tensor.matmul(psy[jj][0][:], lhsT, wy_slice(k, HS[0]), start=st, stop=sp)
                        mm = nc.tensor.matmul(psy[jj][1][:], lhsT, wy_slice(k, HS[1]), start=st, stop=sp)
                    mm_anchor[(phase_idx, k)] = mm.ins
                phase_idx += 1

                # Y epilogue (tail-exposed): softplus, alpha_y,
                # alpha_a = sp2*w1 + alpha_x.
                for jj in range(nrt):
                    r = r0 + jj
                    t_ = epool.tile([P, CLASSES], dt.float32, tag=f"t2_{jj}")
                    nc.vector.tensor_tensor(t_[:, HS[0]], psy[jj][0][:], by_sb[:, HS[0]], ADD)
                    nc.vector.tensor_tensor(t_[:, HS[1]], psy[jj][1][:], by_sb[:, HS[1]], ADD)
                    ay = opool.tile([P, CLASSES], dt.float32, tag=f"ay{jj}")
                    aa = opool.tile([P, CLASSES], dt.float32, tag=f"aa{jj}")
                    for h in range(2):
                        hs = HS[h]
                        sp2 = t_[:, hs]
                        nc.scalar.activation(sp2, sp2, EXP)
                        nc.scalar.activation(sp2, sp2, LN, bias=1.0)
                        nc.scalar.add(ay[:, hs], sp2, 1.0)
                        nc.sync.dma_start(ay3[r][:, hs], ay[:, hs])
                        nc.vector.tensor_tensor(sp2, sp2, t1[jj][:, hs], MULT)
                        nc.vector.tensor_tensor(aa[:, hs], sp2, axt[jj][:, hs], ADD)
                        nc.sync.dma_start(aa3[r][:, hs], aa[:, hs])

            # -- DMA backpressure: gate transfers on PE progress -----------
            def _gate(dma, phase, k, why):
                add_dep_helper(dma.ins, mm_anchor[(phase, min(k, KCH - 1))], reason=why)

            for kk in range(KCH // 2):
                if kk >= 3:
                    _gate(x0_dma[kk], 0, 2 * kk - 6, "x ramp stage")
                    _gate(wx_dma[kk], 0, 2 * kk - 6, "wx ramp stage")
                _gate(y0_dma[kk], 0, min(2 * kk + 2, KCH - 1), "y ramp stage")
                _gate(wy_dma[kk], 0, min(2 * kk + 2, KCH - 1), "wy ramp stage")
                _gate(x1_dma[kk], 1, 2 * kk, "x bulk stage")
                _gate(y1_dma[kk], 2, 2 * kk, "y bulk stage")
            _gate(bx_dma, 0, 10, "bias x stage")
            _gate(by_dma, 0, 12, "bias y stage")

    _split_waits(nc)
    return nc


def kernel(x, y, Wx, bx, Wy, by):
    global LAST_RESULTS
    from concourse.bass_utils import run_bass_kernel_spmd

    if "nc" not in _CACHE:
        _CACHE["nc"] = _build_nc()
    nc = _CACHE["nc"]

    bf16 = ml_dtypes.bfloat16
    x = np.asarray(x, dtype=np.float32)
    y = np.asarray(y, dtype=np.float32)
    xb = x.astype(bf16)                       # [BATCH, DIM]
    yb = y.astype(bf16)
    wxT = np.ascontiguousarray(np.asarray(Wx, dtype=np.float32).astype(bf16).T)  # [DIM, CLASSES]
    wyT = np.ascontiguousarray(np.asarray(Wy, dtype=np.float32).astype(bf16).T)
    bxb = np.ascontiguousarray(
        np.broadcast_to(np.asarray(bx, dtype=np.float32).astype(bf16), (P, CLASSES))
    )
    byb = np.ascontiguousarray(
        np.broadcast_to(np.asarray(by, dtype=np.float32).astype(bf16), (P, CLASSES))
    )

    xTb = np.ascontiguousarray(xb.T)          # [DIM, BATCH]
    yTb = np.ascontiguousarray(yb.T)

    in_maps = []
    for c in range(NCORES):
        rs = slice(c * R, (c + 1) * R)
        in_maps.append(
            {
                "xT": np.ascontiguousarray(xTb[:, rs]),
                "yT": np.ascontiguousarray(yTb[:, rs]),
                "wxT": wxT,
                "wyT": wyT,
                "bxb": bxb,
                "byb": byb,
            }
        )

    res = run_bass_kernel_spmd(nc, in_maps, core_ids=list(range(NCORES)))
    LAST_RESULTS = res

    aa = np.concatenate([res.results[c]["alpha_a"] for c in range(NCORES)], axis=0)
    ax = np.concatenate([res.results[c]["alpha_x"] for c in range(NCORES)], axis=0)
    ay = np.concatenate([res.results[c]["alpha_y"] for c in range(NCORES)], axis=0)
    return (aa, ax, ay)
